# revision 40
# baseline (speedup 1.0000x reference)
"""Cross-modal contrastive loss on 8 Trainium2 NeuronCores.

Strategy (B=8192, d=256 hardcoded):
  * Host sorts rows by patient id (same-patient mask becomes a narrow band)
    and quantizes projections to fp8e4 (scaled by 64; adds ~1e-4 rel err,
    well under the 2e-2 gate).
  * Each core owns a 1024-row slice of z_a and streams the full z_t in eight
    1024-column blocks.  Per (row-tile, block) it computes (128, 1024) logits
    with one DoubleRow fp8 matmul per 512-chunk (K=256 folded into the
    virtual 128x256 array, ~216ns/MM warm), then exponentiates with the fixed
    offset C (|logits| <= SCALE, so no running max is needed):
      - 42 tiles/core on the ACT engine: exp activation reading PSUM
        directly, row sums for free via accum_out;
      - 22 tiles/core on the DVE: Schraudolph exp in bf16-code space (one
        dual-scalar tensor_scalar PSUM-f32 -> int16, whose bits ARE the bf16
        exp), then one STT that adds a neighbouring ACT tile (bitcast bf16)
        producing a pair-sum tile + a mixed accum_out row sum that the host
        disentangles;
      - column sums via PE matmuls with one-hot selector weights (moving =
        exp tiles) accumulating into a [8, 512] PSUM tile per 4-block group;
        these are drip-fed two per row-tile so the in-order PE never clumps
        reduction matmuls and starves the exp engines.
  * Redundant weight loads (the legalizer emits one per matmul) are deduped
    into NoOps post-build; band/bandcol work overlaps the input-DMA window.
  * Host combines per-core row/col/band sums in float64 into the scalar loss.

Measured: ~85-87us vs the 103us bf16 baseline; ACT/DVE balanced at ~68us
busy each (exp is PSUM-port-bound at 1 elem/cycle/lane on both engines),
PE ~61us.
"""

import numpy as np
import ml_dtypes

TEMPERATURE = 0.03
SCALE = 1.0 / TEMPERATURE
C = SCALE + 0.01  # fixed logsumexp offset; logits are <= SCALE * (1 + eps)
B = 8192
D = 256
NCORES = 8
ROWS = B // NCORES          # 1024 rows per core
RT = ROWS // 128            # 8 row-tiles per core
NCB = B // 1024             # 8 column blocks of 1024
BF16 = ml_dtypes.bfloat16
FP8 = ml_dtypes.float8_e4m3
QS = 64.0                   # fp8 pre-scale: za_q = fp8(64*za)
LN2 = float(np.log(2.0))
A16 = 128.0 / LN2           # Schraudolph slope in bf16-code space
B16 = 127.0 * 128.0 - 4.5   # offset tuned to zero the mean exp bias
PSCALE = QS * QS            # matmul psum = PSCALE * sim

_CACHE = {}

def _partner_pairs(cb):
    """(dve_r, act_partner_r) pairs for a block; every tile appears exactly
    once across pairs+singles, so one colsum matmul covers each pair.
    Even and odd blocks are staggered so each block-pair iteration feeds
    the ACT and the DVE one tile each (smooth pipelines, and the two psum
    tiles of an iteration drain on different engines in parallel)."""
    if cb in (0, 4):
        return [(2, 1), (6, 5)]
    if cb % 2 == 0:
        return [(2, 1), (4, 3), (6, 5)]
    return [(1, 0), (3, 2), (5, 4)]


def _dve_rs(cb):
    return tuple(r for r, _ in _partner_pairs(cb))


def _singles_rs(cb):
    if cb in (0, 4):
        return (0, 3, 4, 7)
    return (0, 7) if cb % 2 == 0 else (6, 7)


def _install_drain_patch():
    """walrus accepts at most one sync-wait per CTRL instruction, but
    TileContext's exit drain collects one wait per outstanding semaphore.
    Spread the waits across nop instructions, one wait each."""
    import bass_rust
    import concourse.tile as tile_mod
    from concourse.vector_clock import ScopedClock

    if getattr(tile_mod.TileContext, "_drain_patch_installed", False):
        return

    def _patched(self, tick_clock, wait_clock):
        nc = self.nc
        probe = nc.sync.nop(nofuse=True)
        wait_clock.add_sem_waits(
            probe.ins, ScopedClock({None: tick_clock.global_clock})
        )
        si = probe.ins.sync_info
        waits = list(si.on_wait) if si is not None else []
        if len(waits) > 1:
            si.on_wait = waits[:1]
            for w in waits[1:]:
                extra = nc.sync.nop(nofuse=True)
                extra.ins.sync_info = bass_rust.SyncInfo(on_wait=[w], on_update=[])
        nc.sync.drain()
        nc.all_engine_barrier()
        popped = nc._tile_sem_poison_stack.pop()
        assert popped is self._sem_poison
        nc.clear_and_free_semaphores(list(self.sems.allocated().values()))
        nc.all_engine_barrier()

    tile_mod.TileContext._drain_and_barrier = _patched
    tile_mod.TileContext._drain_patch_installed = True


def _split_multi_waits(nc):
    """walrus in this container accepts at most one sync-wait per instruction.
    Hoist extra waits onto same-engine nops inserted just before the
    instruction (engine streams are in-order, so the waits still gate it)."""
    import bass_rust

    n = 0
    for fn in nc.m.functions:
        for bb in fn.blocks:
            insts = list(bb.instructions)
            out = []
            for inst in insts:
                si = inst.sync_info
                if si is not None and len(si.on_wait) > 1:
                    waits = list(si.on_wait)
                    for w in waits[:-1]:
                        n += 1
                        nop = bass_rust.InstNoOp(
                            name=f"I-waitsplit-{n}", ins=[], outs=[]
                        )
                        nop.engine = inst.engine
                        nop.sync_info = bass_rust.SyncInfo(
                            on_wait=[w], on_update=[]
                        )
                        out.append(nop)
                    si.on_wait = waits[-1:]
                out.append(inst)
            if n:
                bb.instructions = out
    return n


def _dedup_ldweights(nc):
    """The tile legalizer emits one Ldweights per matmul even when
    consecutive matmuls use identical weights.  Convert redundant loads to
    NoOps (keeping their sync waits) so the PE skips the reload stall."""
    import bass_rust

    n = 0
    for fn in nc.m.functions:
        for bb in fn.blocks:
            last_sig = None
            out = []
            for inst in bb.instructions:
                if inst.opcode == "Ldweights":
                    sig = str(inst.ins[0])
                    if sig == last_sig:
                        n += 1
                        nop = bass_rust.InstNoOp(
                            name=f"I-ldwdedup-{n}", ins=[], outs=[]
                        )
                        nop.engine = inst.engine
                        nop.sync_info = inst.sync_info
                        out.append(nop)
                        continue
                    last_sig = sig
                out.append(inst)
            if n:
                bb.instructions = out
    return n


def _build_program(PAD, W, Wb, split_waits=True):
    from contextlib import ExitStack
    import concourse.bass as bass
    import concourse.tile as tile
    from concourse import mybir

    _install_drain_patch()

    nc = bass.Bass()
    bf = mybir.dt.bfloat16
    f32 = mybir.dt.float32
    fp8 = mybir.dt.float8e4
    i16 = mybir.dt.int16
    DR = mybir.MatmulPerfMode.DoubleRow

    # Drop preamble memsets for const APs this program never uses (the
    # serial ~0.8us memsets sit ahead of the first DMA trigger).
    drop = ("const-float32-1.0", "const-bfloat16-1.0", "const-uint8-127")
    bb0 = nc.m.functions[0].blocks[0]
    bb0.instructions = [
        i for i in bb0.instructions
        if not (i.opcode == "Memset"
                and any(d in str(i.outs[0]) for d in drop))
    ]

    zaT = nc.declare_dram_parameter("zaT", [128, 2, ROWS], fp8, isOutput=False)
    ztT = nc.declare_dram_parameter("ztT", [NCB, 128, 2, 1024], fp8, isOutput=False)
    ztTb = nc.declare_dram_parameter("ztTb", [128, 2, Wb], fp8, isOutput=False)
    maskb = nc.declare_dram_parameter("maskb", [128, RT, W], fp8, isOutput=False)

    accpa = nc.declare_dram_parameter("accpa", [128, RT * NCB], f32, isOutput=True)
    accpd = nc.declare_dram_parameter("accpd", [128, RT * NCB], f32, isOutput=True)
    bandrow = nc.declare_dram_parameter("bandrow", [128, RT], f32, isOutput=True)
    colP = nc.declare_dram_parameter("colP", [8, 2, 512], f32, isOutput=True)
    bandcol = nc.declare_dram_parameter("bandcol", [4, 512], f32, isOutput=True)

    s1 = A16 * SCALE / PSCALE
    s2 = B16 - A16 * C

    with ExitStack() as ctx:
        tc = ctx.enter_context(tile.TileContext(nc))
        singles = ctx.enter_context(tc.tile_pool(name="singles", bufs=1))
        ztpool = ctx.enter_context(tc.tile_pool(name="ztpool", bufs=4))
        exppool = ctx.enter_context(tc.tile_pool(name="exppool", bufs=24))
        scrpool = ctx.enter_context(tc.tile_pool(name="scrpool", bufs=14))
        bpool = ctx.enter_context(tc.tile_pool(name="bpool", bufs=2))

        biasC = singles.tile([128, 1], f32)
        nc.vector.memset(biasC[:], -C)
        # Dummy activation: pull the ~2.7us exp table load off the critical
        # path (runs while the first DMAs are in flight).
        warm = singles.tile([128, 1], f32)
        nc.scalar.activation(warm[:], biasC[:], mybir.ActivationFunctionType.Exp)

        # one-hot selector weights for the column-sum matmuls
        sel = singles.tile([128, 8, 8], bf)
        nc.vector.memset(sel[:], 0.0)
        for j in range(8):
            nc.vector.memset(sel[:, j, j:j + 1], 1.0)

        # DMA order: band phase (first compute) needs the weights and band
        # windows first, then the first column blocks and the band mask.
        # One consolidated DMA per tensor (each trigger costs ~620ns of
        # serialized Sync time at startup).
        # Startup loads are descriptor-rate-bound (~20ns per partition row),
        # so split each across two HWDGE queues (sync + scalar trigger in
        # parallel) to halve their latency.
        def dma2(dst, src):
            nc.sync.dma_start(dst[0:64], src[0:64])
            nc.scalar.dma_start(dst[64:128], src[64:128])

        zaTs = singles.tile([128, 2, ROWS], fp8)
        dma2(zaTs, zaT)
        ztb0 = ztpool.tile([128, 2, 1024], fp8, tag="ztb", name="ztb0")
        dma2(ztb0, ztT[0])
        ztTb_sb = singles.tile([128, 2, Wb], fp8)
        nc.sync.dma_start(ztTb_sb[:], ztTb[:])
        ztb1 = ztpool.tile([128, 2, 1024], fp8, tag="ztb", name="ztb1")
        nc.sync.dma_start(ztb1[:], ztT[1])
        maskb_sb = singles.tile([128, RT, W], fp8)
        nc.sync.dma_start(maskb_sb[:], maskb[:])


        def za_sl(r):
            return zaTs[:, :, r * 128:(r + 1) * 128]

        acc_a = singles.tile([128, RT * NCB], f32)
        acc_d = singles.tile([128, RT * NCB], f32)
        bandrow_sb = singles.tile([128, RT], f32)
        colS_sb = singles.tile([8, 2, 512], f32)
        bandcol_sb = singles.tile([4, 512], f32)
        bandstack = singles.tile([128, RT, W], bf)

        pmain = ctx.enter_context(
            tc.tile_pool(name="pmain", bufs=3, space="PSUM"))
        pcol = ctx.enter_context(
            tc.tile_pool(name="pcol", bufs=2, space="PSUM"))
        # Warm the PE (HAM un-throttle needs ~3.4us of activity) with dummy
        # matmuls on already-initialized SBUF while the first loads land.
        wtile = singles.tile([128, 512], bf)
        nc.vector.memset(wtile[:], 0.5)
        pwarm = pcol.tile([8, 512], f32, tag="pcol", name="pwarm")
        for _ in range(10):
            nc.tensor.matmul(
                pwarm[:], sel[:, 0, :], wtile[:],
                start=True, stop=True, skip_group_check=True,
            )

        def band_pass(r0):
            pb = pmain.tile([128, 2 * W], f32, tag="pm", name=f"pb{r0}")
            for i in range(2):
                rr = r0 + i
                nc.tensor.matmul(
                    pb[:, i * W:(i + 1) * W],
                    za_sl(rr),
                    ztTb_sb[:, :, rr * 128:rr * 128 + W],
                    start=True, stop=True,
                    perf_mode=DR, skip_group_check=True,
                )
            exp_b = bpool.tile([128, 2 * W], bf, tag="exp_b")
            nc.scalar.activation(
                exp_b[:], pb[:, :2 * W], mybir.ActivationFunctionType.Exp,
                bias=biasC[:], scale=SCALE / PSCALE,
            )
            for i in range(2):
                nc.vector.scalar_tensor_tensor(
                    out=bandstack[:, r0 + i, :],
                    in0=exp_b[:, i * W:(i + 1) * W],
                    scalar=1.0,
                    in1=maskb_sb[:, r0 + i, :],
                    op0=mybir.AluOpType.mult,
                    op1=mybir.AluOpType.mult,
                    accum_out=bandrow_sb[:, r0 + i:r0 + i + 1],
                )

        # Main loop: 8 column blocks of 1024, processed as 4 block-PAIRS so
        # each za weight load serves 4 matmuls.  Per block, DVE tiles (r in
        # {2,4,6}, plus r=7 on B_CBS blocks) take the Schraudolph path; their
        # pass2 (tensor_tensor_reduce) also adds an ACT partner tile, so one
        # colsum matmul covers the pair (the mixed accum_out is fixed up on
        # the host).  Colsum matmuls run chunk-major in deferred batches so
        # the in-order PE never stalls and each selector loads once.
        pcs = {}
        pcs_n = {}
        pcs_tot = {g: sum(2 * (len(_partner_pairs(cb)) + len(_singles_rs(cb)))
                          for cb in range(4 * g, 4 * g + 4)) for g in range(2)}
        batches = {}  # cb -> [eap, ...]

        col_q = []  # (group, sel_idx, chunk-AP) drip queue

        def flush_batch(cb):
            entries = batches.pop(cb, [])
            g = cb // 4
            for ch in range(2):
                for eap in entries:
                    col_q.append((g, 2 * (cb % 4) + ch,
                                  eap[:, 512 * ch:512 * (ch + 1)]))

        def drip_colsum(nmax):
            while col_q and nmax > 0:
                g, j, chunk = col_q.pop(0)
                nc.tensor.matmul(
                    pcs[g][:],
                    sel[:, j, :],
                    chunk,
                    start=(pcs_n[g] == 0),
                    stop=(pcs_n[g] == pcs_tot[g] - 1),
                    skip_group_check=True,
                )
                pcs_n[g] += 1
                nmax -= 1
                if pcs_n[g] == pcs_tot[g]:
                    nc.vector.tensor_copy(colS_sb[:, g, :], pcs[g][:])
                    nc.sync.dma_start(colP[:, g, :], colS_sb[:, g, :])

        for cb in range(NCB):
            g = cb // 4
            if cb % 4 == 0:
                pcs[g] = pcol.tile([8, 512], f32, tag="pcol", name=f"pcs{g}")
                pcs_n[g] = 0
            if cb == 0:
                ztb = ztb0
            elif cb == 1:
                ztb = ztb1
            else:
                ztb = ztpool.tile([128, 2, 1024], fp8, tag="ztb")
                nc.sync.dma_start(ztb[:], ztT[cb])
            batches[cb] = []
            partner = None
            for r in range(RT):
                if r == 2 and cb > 0:
                    flush_batch(cb - 1)  # previous block's tail colsums
                if r == 2 and cb == 2:
                    # Band column sums: selector matmuls over the band stack
                    # (ready since the band phase; issued here so the PE
                    # stream never waits on them).
                    bs_flat = bandstack[:].rearrange("p r w -> p (r w)")
                    pbc = pmain.tile([4, 512], f32, tag="pm", name="pbc")
                    for ch in range(4):
                        nc.tensor.matmul(
                            pbc[:], sel[:, ch, 0:4],
                            bs_flat[:, 512 * ch:512 * (ch + 1)],
                            start=(ch == 0), stop=(ch == 3),
                            skip_group_check=True,
                        )
                    nc.vector.tensor_copy(bandcol_sb[:], pbc[:])
                    nc.sync.dma_start(bandcol[:], bandcol_sb[:])
                if r == 6:
                    flush_batch(cb)  # first colsum targets of this block
                    batches[cb] = []
                if cb == 1 and r % 2 == 0:
                    band_pass(2 * (r // 2))  # band overlaps the early blocks
                if cb == 3 and r == 0:
                    nc.sync.dma_start(bandrow[:], bandrow_sb[:])
                if cb == 5 and r == 0:
                    half = 4 * RT
                    nc.sync.dma_start(accpa[:, 0:half], acc_a[:, 0:half])
                    nc.sync.dma_start(accpd[:, 0:half], acc_d[:, 0:half])
                pm = pmain.tile([128, 1024], f32, tag="pm")
                for ch in range(2):
                    nc.tensor.matmul(
                        pm[:, ch * 512:(ch + 1) * 512],
                        za_sl(r),
                        ztb[:, :, ch * 512:(ch + 1) * 512],
                        start=True, stop=True,
                        perf_mode=DR, skip_group_check=True,
                    )
                drip_colsum(3)
                aidx = cb * RT + r
                if r in _dve_rs(cb):
                    code = exppool.tile([128, 1024], i16, tag="exp")
                    nc.vector.tensor_scalar(
                        out=code[:], in0=pm[:], scalar1=s1, scalar2=s2,
                        op0=mybir.AluOpType.mult, op1=mybir.AluOpType.add,
                    )
                    scr = scrpool.tile([128, 1024], bf, tag="scr")
                    nc.vector.scalar_tensor_tensor(
                        out=scr[:], in0=partner[:], scalar=1.0,
                        in1=code[:].bitcast(bf),
                        op0=mybir.AluOpType.mult,
                        op1=mybir.AluOpType.add,
                        accum_out=acc_d[:, aidx:aidx + 1],
                    )
                    batches[cb].append(scr[:])
                else:
                    ex = exppool.tile([128, 1024], bf, tag="exp")
                    nc.scalar.activation(
                        ex[:], pm[:], mybir.ActivationFunctionType.Exp,
                        bias=biasC[:], scale=SCALE / PSCALE,
                        accum_out=acc_a[:, aidx:aidx + 1],
                    )
                    partner = ex
                    if r in _singles_rs(cb):
                        batches[cb].append(ex[:])
        flush_batch(NCB - 1)
        drip_colsum(len(col_q))

        half = 4 * RT
        nc.sync.dma_start(accpa[:, half:], acc_a[:, half:])
        nc.sync.dma_start(accpd[:, half:], acc_d[:, half:])

    if split_waits:
        _split_multi_waits(nc)
    _dedup_ldweights(nc)
    return nc


def _prep_inputs(za8, zt8, pid_s, PAD, W, Wb):
    """Build the per-core input maps."""
    zt8T = np.ascontiguousarray(zt8.T)  # (256, 8192)
    ztT_all = np.ascontiguousarray(
        zt8T.reshape(2, 128, NCB, 1024).transpose(2, 1, 0, 3)
    )  # (NCB, 128, 2, 1024)

    pidp = np.full(B + 2 * PAD, -1, dtype=np.int64)
    pidp[PAD:PAD + B] = pid_s
    zt8T_pad = np.zeros((D, B + 2 * PAD), dtype=FP8)
    zt8T_pad[:, PAD:PAD + B] = zt8T

    in_maps = []
    for c in range(NCORES):
        r0 = c * ROWS
        zaTc = np.ascontiguousarray(
            za8[r0:r0 + ROWS].T.reshape(2, 128, ROWS).transpose(1, 0, 2)
        )  # (128, 2, ROWS)
        band = zt8T_pad[:, r0:r0 + Wb]  # global cols [r0-PAD, r0+Wb-PAD)
        ztTbc = np.ascontiguousarray(band.reshape(2, 128, Wb).transpose(1, 0, 2))
        mask = np.zeros((128, RT, W), dtype=FP8)
        for r in range(RT):
            rows = pid_s[r0 + r * 128: r0 + (r + 1) * 128]
            cols = pidp[r0 + r * 128: r0 + r * 128 + W]  # starts at global-PAD
            mask[:, r, :] = (rows[:, None] == cols[None, :]).astype(FP8)
        in_maps.append({"zaT": zaTc, "ztT": ztT_all, "ztTb": ztTbc, "maskb": mask})
    return in_maps


def _numpy_fallback(z_a, z_t, patient_ids):
    z_a = np.asarray(z_a, np.float64)
    z_t = np.asarray(z_t, np.float64)
    pid = np.asarray(patient_ids)
    sim = (z_a @ z_t.T) / TEMPERATURE
    cross = pid[:, None] != pid[None, :]

    def direction(sim, cross):
        n = sim.shape[0]
        pos = np.diagonal(sim)
        mask = cross | np.eye(n, dtype=bool)
        neg = np.where(mask, sim, -np.inf)
        m = neg.max(axis=1)
        lse = np.log(np.exp(neg - m[:, None]).sum(axis=1)) + m
        row_loss = lse - pos
        valid = cross.any(axis=1)
        cnt = valid.sum()
        return (row_loss[valid].sum() / cnt) if cnt > 0 else 0.0

    loss = 0.5 * (direction(sim, cross) + direction(sim.T, cross.T))
    return np.asarray(loss, dtype=np.float32)


def kernel(z_a, z_t, patient_ids):
    from concourse.bass_utils import run_bass_kernel_spmd

    z_a = np.asarray(z_a)
    z_t = np.asarray(z_t)
    pid = np.asarray(patient_ids)
    assert z_a.shape == (B, D) and z_t.shape == (B, D)

    # Sort rows by patient id so same-patient pairs live in a diagonal band.
    perm = np.argsort(pid, kind="stable")
    pid_s = pid[perm].astype(np.int64)
    za_s = z_a[perm]
    zt_s = z_t[perm]

    _, counts = np.unique(pid_s, return_counts=True)
    gmax = int(counts.max())
    if gmax > 64:
        return _numpy_fallback(z_a, z_t, patient_ids)
    PAD, W = 64, 256
    Wb = ROWS + 2 * PAD

    za8 = (za_s.astype(np.float32) * QS).astype(FP8)
    zt8 = (zt_s.astype(np.float32) * QS).astype(FP8)

    key = (PAD, W, Wb)
    if key not in _CACHE:
        _CACHE[key] = _build_program(PAD, W, Wb)
    nc = _CACHE[key]

    in_maps = _prep_inputs(za8, zt8, pid_s, PAD, W, Wb)
    r = run_bass_kernel_spmd(nc, in_maps, list(range(NCORES)))
    global _LAST_RESULT
    _LAST_RESULT = r
    res = r.results

    # Host-side assembly in float64.
    zaq = za8.astype(np.float64) / QS
    ztq = zt8.astype(np.float64) / QS
    pos = (zaq * ztq).sum(axis=1) * SCALE
    pos_exp = np.exp(pos - C)

    # Row sums from the raw per-(r, cb) accumulators.  DVE tiles' accum also
    # contains their ACT partner tile's row sum (the pass2 pair-add), so
    # subtract the partner's accumulator row.
    S_parts = []
    for c in range(NCORES):
        aa = res[c]["accpa"].astype(np.float64).reshape(
            128, NCB, RT).transpose(0, 2, 1)
        ad = res[c]["accpd"].astype(np.float64).reshape(
            128, NCB, RT).transpose(0, 2, 1)
        dmask = np.zeros((RT, NCB), dtype=bool)
        for cb_ in range(NCB):
            for rd_ in _dve_rs(cb_):
                dmask[rd_, cb_] = True
        a = np.where(dmask[None, :, :], ad, aa)
        rs = a.sum(axis=2)  # (128, RT)
        for cb in range(NCB):
            for rd, rp in _partner_pairs(cb):
                rs[:, rd] -= a[:, rp, cb]
        S_parts.append(rs.T.reshape(-1))  # r-major then partition
    S_all = np.concatenate(S_parts)
    B_row = np.concatenate(
        [res[c]["bandrow"].T.reshape(-1) for c in range(NCORES)]
    ).astype(np.float64)
    colS = np.zeros(B, dtype=np.float64)
    for c in range(NCORES):
        cp = res[c]["colP"].astype(np.float64)  # [8, 2, 512]
        for cb in range(NCB):
            g, p0 = cb // 4, 2 * (cb % 4)
            for ch in range(2):
                colS[cb * 1024 + ch * 512: cb * 1024 + (ch + 1) * 512] += \
                    cp[p0 + ch, g, :]
    B_col = np.zeros(B, dtype=np.float64)
    for c in range(NCORES):
        bc = res[c]["bandcol"].astype(np.float64).reshape(RT, W)
        for r_ in range(RT):
            g0 = c * ROWS + r_ * 128 - PAD  # global col of band idx 0
            lo = max(0, -g0)
            hi = min(W, B - g0)
            B_col[g0 + lo:g0 + hi] += bc[r_, lo:hi]

    Sa = np.maximum(S_all - B_row + pos_exp, 1e-300)
    St = np.maximum(colS - B_col + pos_exp, 1e-300)
    row_loss_a = C + np.log(Sa) - pos
    row_loss_t = C + np.log(St) - pos

    uniq, inv, cnts = np.unique(pid_s, return_inverse=True, return_counts=True)
    group_sizes = cnts[inv]
    valid = group_sizes < B
    cnt = int(valid.sum())
    if cnt > 0:
        loss_a = row_loss_a[valid].sum() / cnt
        loss_t = row_loss_t[valid].sum() / cnt
    else:
        loss_a = loss_t = 0.0

    return np.asarray((loss_a + loss_t) / 2.0, dtype=np.float32)


# revision 41
# speedup vs baseline: 1.0260x; 1.0260x over previous
"""Cross-modal contrastive loss on 8 Trainium2 NeuronCores.

Strategy (B=8192, d=256 hardcoded):
  * Host sorts rows by patient id (same-patient mask becomes a narrow band)
    and quantizes projections to fp8e4 (scaled by 64; adds ~1e-4 rel err,
    well under the 2e-2 gate).
  * Each core owns a 1024-row slice of z_a and streams the full z_t in eight
    1024-column blocks.  Per (row-tile, block) it computes (128, 1024) logits
    with one DoubleRow fp8 matmul per 512-chunk (K=256 folded into the
    virtual 128x256 array, ~216ns/MM warm), then exponentiates with the fixed
    offset C (|logits| <= SCALE, so no running max is needed):
      - 42 tiles/core on the ACT engine: exp activation reading PSUM
        directly, row sums for free via accum_out;
      - 22 tiles/core on the DVE: Schraudolph exp in bf16-code space (one
        dual-scalar tensor_scalar PSUM-f32 -> int16, whose bits ARE the bf16
        exp), then one STT that adds a neighbouring ACT tile (bitcast bf16)
        producing a pair-sum tile + a mixed accum_out row sum that the host
        disentangles;
      - column sums via PE matmuls with one-hot selector weights (moving =
        exp tiles) accumulating into a [8, 512] PSUM tile per 4-block group;
        these are drip-fed two per row-tile so the in-order PE never clumps
        reduction matmuls and starves the exp engines.
  * Redundant weight loads (the legalizer emits one per matmul) are deduped
    into NoOps post-build; band/bandcol work overlaps the input-DMA window.
  * Host combines per-core row/col/band sums in float64 into the scalar loss.

Measured: ~85-87us vs the 103us bf16 baseline; ACT/DVE balanced at ~68us
busy each (exp is PSUM-port-bound at 1 elem/cycle/lane on both engines),
PE ~61us.
"""

import numpy as np
import ml_dtypes

TEMPERATURE = 0.03
SCALE = 1.0 / TEMPERATURE
C = SCALE + 0.01  # fixed logsumexp offset; logits are <= SCALE * (1 + eps)
B = 8192
D = 256
NCORES = 8
ROWS = B // NCORES          # 1024 rows per core
RT = ROWS // 128            # 8 row-tiles per core
NCB = B // 1024             # 8 column blocks of 1024
BF16 = ml_dtypes.bfloat16
FP8 = ml_dtypes.float8_e4m3
QS = 64.0                   # fp8 pre-scale: za_q = fp8(64*za)
LN2 = float(np.log(2.0))
A16 = 128.0 / LN2           # Schraudolph slope in bf16-code space
B16 = 127.0 * 128.0 - 4.5   # offset tuned to zero the mean exp bias
PSCALE = QS * QS            # matmul psum = PSCALE * sim

_CACHE = {}

def _partner_pairs(cb):
    """(dve_r, act_partner_r) pairs for a block; every tile appears exactly
    once across pairs+singles, so one colsum matmul covers each pair.
    Even and odd blocks are staggered so each block-pair iteration feeds
    the ACT and the DVE one tile each (smooth pipelines, and the two psum
    tiles of an iteration drain on different engines in parallel)."""
    if cb in (0, 4):
        return [(2, 1), (6, 5)]
    if cb % 2 == 0:
        return [(2, 1), (4, 3), (6, 5)]
    return [(1, 0), (3, 2), (5, 4)]


def _dve_rs(cb):
    return tuple(r for r, _ in _partner_pairs(cb))


def _singles_rs(cb):
    if cb in (0, 4):
        return (0, 3, 4, 7)
    return (0, 7) if cb % 2 == 0 else (6, 7)


def _install_drain_patch():
    """walrus accepts at most one sync-wait per CTRL instruction, but
    TileContext's exit drain collects one wait per outstanding semaphore.
    Spread the waits across nop instructions, one wait each."""
    import bass_rust
    import concourse.tile as tile_mod
    from concourse.vector_clock import ScopedClock

    if getattr(tile_mod.TileContext, "_drain_patch_installed", False):
        return

    def _patched(self, tick_clock, wait_clock):
        nc = self.nc
        probe = nc.sync.nop(nofuse=True)
        wait_clock.add_sem_waits(
            probe.ins, ScopedClock({None: tick_clock.global_clock})
        )
        si = probe.ins.sync_info
        waits = list(si.on_wait) if si is not None else []
        if len(waits) > 1:
            si.on_wait = waits[:1]
            for w in waits[1:]:
                extra = nc.sync.nop(nofuse=True)
                extra.ins.sync_info = bass_rust.SyncInfo(on_wait=[w], on_update=[])
        nc.sync.drain()
        nc.all_engine_barrier()
        popped = nc._tile_sem_poison_stack.pop()
        assert popped is self._sem_poison
        nc.clear_and_free_semaphores(list(self.sems.allocated().values()))
        nc.all_engine_barrier()

    tile_mod.TileContext._drain_and_barrier = _patched
    tile_mod.TileContext._drain_patch_installed = True


def _split_multi_waits(nc):
    """walrus in this container accepts at most one sync-wait per instruction.
    Hoist extra waits onto same-engine nops inserted just before the
    instruction (engine streams are in-order, so the waits still gate it)."""
    import bass_rust

    n = 0
    for fn in nc.m.functions:
        for bb in fn.blocks:
            insts = list(bb.instructions)
            out = []
            for inst in insts:
                si = inst.sync_info
                if si is not None and len(si.on_wait) > 1:
                    waits = list(si.on_wait)
                    for w in waits[:-1]:
                        n += 1
                        nop = bass_rust.InstNoOp(
                            name=f"I-waitsplit-{n}", ins=[], outs=[]
                        )
                        nop.engine = inst.engine
                        nop.sync_info = bass_rust.SyncInfo(
                            on_wait=[w], on_update=[]
                        )
                        out.append(nop)
                    si.on_wait = waits[-1:]
                out.append(inst)
            if n:
                bb.instructions = out
    return n


def _dedup_ldweights(nc):
    """The tile legalizer emits one Ldweights per matmul even when
    consecutive matmuls use identical weights.  Convert redundant loads to
    NoOps (keeping their sync waits) so the PE skips the reload stall."""
    import bass_rust

    n = 0
    for fn in nc.m.functions:
        for bb in fn.blocks:
            last_sig = None
            out = []
            for inst in bb.instructions:
                if inst.opcode == "Ldweights":
                    sig = str(inst.ins[0])
                    if sig == last_sig:
                        n += 1
                        nop = bass_rust.InstNoOp(
                            name=f"I-ldwdedup-{n}", ins=[], outs=[]
                        )
                        nop.engine = inst.engine
                        nop.sync_info = inst.sync_info
                        out.append(nop)
                        continue
                    last_sig = sig
                out.append(inst)
            if n:
                bb.instructions = out
    return n


def _build_program(PAD, W, Wb, split_waits=True):
    from contextlib import ExitStack
    import concourse.bass as bass
    import concourse.tile as tile
    from concourse import mybir

    _install_drain_patch()

    nc = bass.Bass()
    bf = mybir.dt.bfloat16
    f32 = mybir.dt.float32
    fp8 = mybir.dt.float8e4
    i16 = mybir.dt.int16
    DR = mybir.MatmulPerfMode.DoubleRow

    # Drop preamble memsets for const APs this program never uses (the
    # serial ~0.8us memsets sit ahead of the first DMA trigger).
    drop = ("const-float32-1.0", "const-bfloat16-1.0", "const-uint8-127")
    bb0 = nc.m.functions[0].blocks[0]
    bb0.instructions = [
        i for i in bb0.instructions
        if not (i.opcode == "Memset"
                and any(d in str(i.outs[0]) for d in drop))
    ]

    zaT = nc.declare_dram_parameter("zaT", [128, 2, ROWS], fp8, isOutput=False)
    ztT = nc.declare_dram_parameter("ztT", [NCB, 128, 2, 1024], fp8, isOutput=False)
    ztTb = nc.declare_dram_parameter("ztTb", [128, 2, Wb], fp8, isOutput=False)
    maskb = nc.declare_dram_parameter("maskb", [128, RT, W], fp8, isOutput=False)

    accpa = nc.declare_dram_parameter("accpa", [128, RT * NCB], f32, isOutput=True)
    accpd = nc.declare_dram_parameter("accpd", [128, RT * NCB], f32, isOutput=True)
    bandrow = nc.declare_dram_parameter("bandrow", [128, RT], f32, isOutput=True)
    colP = nc.declare_dram_parameter("colP", [8, 2, 512], f32, isOutput=True)
    bandcol = nc.declare_dram_parameter("bandcol", [4, 512], f32, isOutput=True)

    s1 = A16 * SCALE / PSCALE
    s2 = B16 - A16 * C

    with ExitStack() as ctx:
        tc = ctx.enter_context(tile.TileContext(nc))
        singles = ctx.enter_context(tc.tile_pool(name="singles", bufs=1))
        ztpool = ctx.enter_context(tc.tile_pool(name="ztpool", bufs=4))
        exppool = ctx.enter_context(tc.tile_pool(name="exppool", bufs=24))
        scrpool = ctx.enter_context(tc.tile_pool(name="scrpool", bufs=14))
        bpool = ctx.enter_context(tc.tile_pool(name="bpool", bufs=2))

        biasC = singles.tile([128, 1], f32)
        nc.vector.memset(biasC[:], -C)
        # Dummy activation: pull the ~2.7us exp table load off the critical
        # path (runs while the first DMAs are in flight).
        warm = singles.tile([128, 1], f32)
        nc.scalar.activation(warm[:], biasC[:], mybir.ActivationFunctionType.Exp)

        # one-hot selector weights for the column-sum matmuls
        sel = singles.tile([128, 8, 8], bf)
        nc.vector.memset(sel[:], 0.0)
        for j in range(8):
            nc.vector.memset(sel[:, j, j:j + 1], 1.0)

        # DMA order: band phase (first compute) needs the weights and band
        # windows first, then the first column blocks and the band mask.
        # One consolidated DMA per tensor (each trigger costs ~620ns of
        # serialized Sync time at startup).
        # Startup loads are descriptor-rate-bound (~20ns per partition row),
        # so split each across two HWDGE queues (sync + scalar trigger in
        # parallel) to halve their latency.
        def dma2(dst, src):
            nc.sync.dma_start(dst[0:64], src[0:64])
            nc.scalar.dma_start(dst[64:128], src[64:128])

        zaTs = singles.tile([128, 2, ROWS], fp8)
        dma2(zaTs, zaT)
        ztb0 = ztpool.tile([128, 2, 1024], fp8, tag="ztb", name="ztb0")
        dma2(ztb0, ztT[0])
        ztTb_sb = singles.tile([128, 2, Wb], fp8)
        dma2(ztTb_sb, ztTb)
        ztb1 = ztpool.tile([128, 2, 1024], fp8, tag="ztb", name="ztb1")
        dma2(ztb1, ztT[1])
        maskb_sb = singles.tile([128, RT, W], fp8)
        dma2(maskb_sb, maskb)


        def za_sl(r):
            return zaTs[:, :, r * 128:(r + 1) * 128]

        acc_a = singles.tile([128, RT * NCB], f32)
        acc_d = singles.tile([128, RT * NCB], f32)
        bandrow_sb = singles.tile([128, RT], f32)
        colS_sb = singles.tile([8, 2, 512], f32)
        bandcol_sb = singles.tile([4, 512], f32)
        bandstack = singles.tile([128, RT, W], bf)

        pmain = ctx.enter_context(
            tc.tile_pool(name="pmain", bufs=3, space="PSUM"))
        pcol = ctx.enter_context(
            tc.tile_pool(name="pcol", bufs=2, space="PSUM"))
        # Warm the PE (HAM un-throttle needs ~3.4us of activity) with dummy
        # matmuls on already-initialized SBUF while the first loads land.
        wtile = singles.tile([128, 512], bf)
        nc.vector.memset(wtile[:], 0.5)
        pwarm = pcol.tile([8, 512], f32, tag="pcol", name="pwarm")
        for _ in range(10):
            nc.tensor.matmul(
                pwarm[:], sel[:, 0, :], wtile[:],
                start=True, stop=True, skip_group_check=True,
            )

        def band_pass(r0):
            pb = pmain.tile([128, 2 * W], f32, tag="pm", name=f"pb{r0}")
            for i in range(2):
                rr = r0 + i
                nc.tensor.matmul(
                    pb[:, i * W:(i + 1) * W],
                    za_sl(rr),
                    ztTb_sb[:, :, rr * 128:rr * 128 + W],
                    start=True, stop=True,
                    perf_mode=DR, skip_group_check=True,
                )
            exp_b = bpool.tile([128, 2 * W], bf, tag="exp_b")
            nc.scalar.activation(
                exp_b[:], pb[:, :2 * W], mybir.ActivationFunctionType.Exp,
                bias=biasC[:], scale=SCALE / PSCALE,
            )
            for i in range(2):
                nc.vector.scalar_tensor_tensor(
                    out=bandstack[:, r0 + i, :],
                    in0=exp_b[:, i * W:(i + 1) * W],
                    scalar=1.0,
                    in1=maskb_sb[:, r0 + i, :],
                    op0=mybir.AluOpType.mult,
                    op1=mybir.AluOpType.mult,
                    accum_out=bandrow_sb[:, r0 + i:r0 + i + 1],
                )

        # Main loop: 8 column blocks of 1024, processed as 4 block-PAIRS so
        # each za weight load serves 4 matmuls.  Per block, DVE tiles (r in
        # {2,4,6}, plus r=7 on B_CBS blocks) take the Schraudolph path; their
        # pass2 (tensor_tensor_reduce) also adds an ACT partner tile, so one
        # colsum matmul covers the pair (the mixed accum_out is fixed up on
        # the host).  Colsum matmuls run chunk-major in deferred batches so
        # the in-order PE never stalls and each selector loads once.
        pcs = {}
        pcs_n = {}
        pcs_tot = {g: sum(2 * (len(_partner_pairs(cb)) + len(_singles_rs(cb)))
                          for cb in range(4 * g, 4 * g + 4)) for g in range(2)}
        batches = {}  # cb -> [eap, ...]

        col_q = []  # (group, sel_idx, chunk-AP) drip queue

        def flush_batch(cb):
            entries = batches.pop(cb, [])
            g = cb // 4
            for ch in range(2):
                for eap in entries:
                    col_q.append((g, 2 * (cb % 4) + ch,
                                  eap[:, 512 * ch:512 * (ch + 1)]))

        def drip_colsum(nmax):
            while col_q and nmax > 0:
                g, j, chunk = col_q.pop(0)
                nc.tensor.matmul(
                    pcs[g][:],
                    sel[:, j, :],
                    chunk,
                    start=(pcs_n[g] == 0),
                    stop=(pcs_n[g] == pcs_tot[g] - 1),
                    skip_group_check=True,
                )
                pcs_n[g] += 1
                nmax -= 1
                if pcs_n[g] == pcs_tot[g]:
                    nc.vector.tensor_copy(colS_sb[:, g, :], pcs[g][:])
                    nc.sync.dma_start(colP[:, g, :], colS_sb[:, g, :])

        for cb in range(NCB):
            g = cb // 4
            if cb % 4 == 0:
                pcs[g] = pcol.tile([8, 512], f32, tag="pcol", name=f"pcs{g}")
                pcs_n[g] = 0
            if cb == 0:
                ztb = ztb0
            elif cb == 1:
                ztb = ztb1
            else:
                ztb = ztpool.tile([128, 2, 1024], fp8, tag="ztb")
                nc.sync.dma_start(ztb[:], ztT[cb])
            batches[cb] = []
            partner = None
            for r in range(RT):
                if r == 2 and cb > 0:
                    flush_batch(cb - 1)  # previous block's tail colsums
                if r == 2 and cb == 2:
                    # Band column sums: selector matmuls over the band stack
                    # (ready since the band phase; issued here so the PE
                    # stream never waits on them).
                    bs_flat = bandstack[:].rearrange("p r w -> p (r w)")
                    pbc = pmain.tile([4, 512], f32, tag="pm", name="pbc")
                    for ch in range(4):
                        nc.tensor.matmul(
                            pbc[:], sel[:, ch, 0:4],
                            bs_flat[:, 512 * ch:512 * (ch + 1)],
                            start=(ch == 0), stop=(ch == 3),
                            skip_group_check=True,
                        )
                    nc.vector.tensor_copy(bandcol_sb[:], pbc[:])
                    nc.sync.dma_start(bandcol[:], bandcol_sb[:])
                if r == 6:
                    flush_batch(cb)  # first colsum targets of this block
                    batches[cb] = []
                if cb == 1 and r % 2 == 0:
                    band_pass(2 * (r // 2))  # band overlaps the early blocks
                if cb == 3 and r == 0:
                    nc.sync.dma_start(bandrow[:], bandrow_sb[:])
                if cb == 5 and r == 0:
                    half = 4 * RT
                    nc.sync.dma_start(accpa[:, 0:half], acc_a[:, 0:half])
                    nc.sync.dma_start(accpd[:, 0:half], acc_d[:, 0:half])
                pm = pmain.tile([128, 1024], f32, tag="pm")
                for ch in range(2):
                    nc.tensor.matmul(
                        pm[:, ch * 512:(ch + 1) * 512],
                        za_sl(r),
                        ztb[:, :, ch * 512:(ch + 1) * 512],
                        start=True, stop=True,
                        perf_mode=DR, skip_group_check=True,
                    )
                drip_colsum(3)
                aidx = cb * RT + r
                if r in _dve_rs(cb):
                    code = exppool.tile([128, 1024], i16, tag="exp")
                    nc.vector.tensor_scalar(
                        out=code[:], in0=pm[:], scalar1=s1, scalar2=s2,
                        op0=mybir.AluOpType.mult, op1=mybir.AluOpType.add,
                    )
                    scr = scrpool.tile([128, 1024], bf, tag="scr")
                    nc.vector.scalar_tensor_tensor(
                        out=scr[:], in0=partner[:], scalar=1.0,
                        in1=code[:].bitcast(bf),
                        op0=mybir.AluOpType.mult,
                        op1=mybir.AluOpType.add,
                        accum_out=acc_d[:, aidx:aidx + 1],
                    )
                    batches[cb].append(scr[:])
                else:
                    ex = exppool.tile([128, 1024], bf, tag="exp")
                    nc.scalar.activation(
                        ex[:], pm[:], mybir.ActivationFunctionType.Exp,
                        bias=biasC[:], scale=SCALE / PSCALE,
                        accum_out=acc_a[:, aidx:aidx + 1],
                    )
                    partner = ex
                    if r in _singles_rs(cb):
                        batches[cb].append(ex[:])
        flush_batch(NCB - 1)
        drip_colsum(len(col_q))

        half = 4 * RT
        nc.sync.dma_start(accpa[:, half:], acc_a[:, half:])
        nc.sync.dma_start(accpd[:, half:], acc_d[:, half:])

    if split_waits:
        _split_multi_waits(nc)
    _dedup_ldweights(nc)
    return nc


def _prep_inputs(za8, zt8, pid_s, PAD, W, Wb):
    """Build the per-core input maps."""
    zt8T = np.ascontiguousarray(zt8.T)  # (256, 8192)
    ztT_all = np.ascontiguousarray(
        zt8T.reshape(2, 128, NCB, 1024).transpose(2, 1, 0, 3)
    )  # (NCB, 128, 2, 1024)

    pidp = np.full(B + 2 * PAD, -1, dtype=np.int64)
    pidp[PAD:PAD + B] = pid_s
    zt8T_pad = np.zeros((D, B + 2 * PAD), dtype=FP8)
    zt8T_pad[:, PAD:PAD + B] = zt8T

    in_maps = []
    for c in range(NCORES):
        r0 = c * ROWS
        zaTc = np.ascontiguousarray(
            za8[r0:r0 + ROWS].T.reshape(2, 128, ROWS).transpose(1, 0, 2)
        )  # (128, 2, ROWS)
        band = zt8T_pad[:, r0:r0 + Wb]  # global cols [r0-PAD, r0+Wb-PAD)
        ztTbc = np.ascontiguousarray(band.reshape(2, 128, Wb).transpose(1, 0, 2))
        mask = np.zeros((128, RT, W), dtype=FP8)
        for r in range(RT):
            rows = pid_s[r0 + r * 128: r0 + (r + 1) * 128]
            cols = pidp[r0 + r * 128: r0 + r * 128 + W]  # starts at global-PAD
            mask[:, r, :] = (rows[:, None] == cols[None, :]).astype(FP8)
        in_maps.append({"zaT": zaTc, "ztT": ztT_all, "ztTb": ztTbc, "maskb": mask})
    return in_maps


def _numpy_fallback(z_a, z_t, patient_ids):
    z_a = np.asarray(z_a, np.float64)
    z_t = np.asarray(z_t, np.float64)
    pid = np.asarray(patient_ids)
    sim = (z_a @ z_t.T) / TEMPERATURE
    cross = pid[:, None] != pid[None, :]

    def direction(sim, cross):
        n = sim.shape[0]
        pos = np.diagonal(sim)
        mask = cross | np.eye(n, dtype=bool)
        neg = np.where(mask, sim, -np.inf)
        m = neg.max(axis=1)
        lse = np.log(np.exp(neg - m[:, None]).sum(axis=1)) + m
        row_loss = lse - pos
        valid = cross.any(axis=1)
        cnt = valid.sum()
        return (row_loss[valid].sum() / cnt) if cnt > 0 else 0.0

    loss = 0.5 * (direction(sim, cross) + direction(sim.T, cross.T))
    return np.asarray(loss, dtype=np.float32)


def kernel(z_a, z_t, patient_ids):
    from concourse.bass_utils import run_bass_kernel_spmd

    z_a = np.asarray(z_a)
    z_t = np.asarray(z_t)
    pid = np.asarray(patient_ids)
    assert z_a.shape == (B, D) and z_t.shape == (B, D)

    # Sort rows by patient id so same-patient pairs live in a diagonal band.
    perm = np.argsort(pid, kind="stable")
    pid_s = pid[perm].astype(np.int64)
    za_s = z_a[perm]
    zt_s = z_t[perm]

    _, counts = np.unique(pid_s, return_counts=True)
    gmax = int(counts.max())
    if gmax > 64:
        return _numpy_fallback(z_a, z_t, patient_ids)
    PAD, W = 64, 256
    Wb = ROWS + 2 * PAD

    za8 = (za_s.astype(np.float32) * QS).astype(FP8)
    zt8 = (zt_s.astype(np.float32) * QS).astype(FP8)

    key = (PAD, W, Wb)
    if key not in _CACHE:
        _CACHE[key] = _build_program(PAD, W, Wb)
    nc = _CACHE[key]

    in_maps = _prep_inputs(za8, zt8, pid_s, PAD, W, Wb)
    r = run_bass_kernel_spmd(nc, in_maps, list(range(NCORES)))
    global _LAST_RESULT
    _LAST_RESULT = r
    res = r.results

    # Host-side assembly in float64.
    zaq = za8.astype(np.float64) / QS
    ztq = zt8.astype(np.float64) / QS
    pos = (zaq * ztq).sum(axis=1) * SCALE
    pos_exp = np.exp(pos - C)

    # Row sums from the raw per-(r, cb) accumulators.  DVE tiles' accum also
    # contains their ACT partner tile's row sum (the pass2 pair-add), so
    # subtract the partner's accumulator row.
    S_parts = []
    for c in range(NCORES):
        aa = res[c]["accpa"].astype(np.float64).reshape(
            128, NCB, RT).transpose(0, 2, 1)
        ad = res[c]["accpd"].astype(np.float64).reshape(
            128, NCB, RT).transpose(0, 2, 1)
        dmask = np.zeros((RT, NCB), dtype=bool)
        for cb_ in range(NCB):
            for rd_ in _dve_rs(cb_):
                dmask[rd_, cb_] = True
        a = np.where(dmask[None, :, :], ad, aa)
        rs = a.sum(axis=2)  # (128, RT)
        for cb in range(NCB):
            for rd, rp in _partner_pairs(cb):
                rs[:, rd] -= a[:, rp, cb]
        S_parts.append(rs.T.reshape(-1))  # r-major then partition
    S_all = np.concatenate(S_parts)
    B_row = np.concatenate(
        [res[c]["bandrow"].T.reshape(-1) for c in range(NCORES)]
    ).astype(np.float64)
    colS = np.zeros(B, dtype=np.float64)
    for c in range(NCORES):
        cp = res[c]["colP"].astype(np.float64)  # [8, 2, 512]
        for cb in range(NCB):
            g, p0 = cb // 4, 2 * (cb % 4)
            for ch in range(2):
                colS[cb * 1024 + ch * 512: cb * 1024 + (ch + 1) * 512] += \
                    cp[p0 + ch, g, :]
    B_col = np.zeros(B, dtype=np.float64)
    for c in range(NCORES):
        bc = res[c]["bandcol"].astype(np.float64).reshape(RT, W)
        for r_ in range(RT):
            g0 = c * ROWS + r_ * 128 - PAD  # global col of band idx 0
            lo = max(0, -g0)
            hi = min(W, B - g0)
            B_col[g0 + lo:g0 + hi] += bc[r_, lo:hi]

    Sa = np.maximum(S_all - B_row + pos_exp, 1e-300)
    St = np.maximum(colS - B_col + pos_exp, 1e-300)
    row_loss_a = C + np.log(Sa) - pos
    row_loss_t = C + np.log(St) - pos

    uniq, inv, cnts = np.unique(pid_s, return_inverse=True, return_counts=True)
    group_sizes = cnts[inv]
    valid = group_sizes < B
    cnt = int(valid.sum())
    if cnt > 0:
        loss_a = row_loss_a[valid].sum() / cnt
        loss_t = row_loss_t[valid].sum() / cnt
    else:
        loss_a = loss_t = 0.0

    return np.asarray((loss_a + loss_t) / 2.0, dtype=np.float32)
